# revision 25
# baseline (speedup 1.0000x reference)
"""Trainium2 Bass kernel for CustomAttention (ViT-style windowed attention).

Math (per batch element):
  qkv = x @ qkv_w.T + qkv_b            -> q, k, v  [H=12 heads, D=64]
  s   = (q * D^-0.5) @ k.T             masked by a fixed 24x24-grid window
  attn = softmax(s)                    (CLS row/col always attended)
  out  = attn @ v                      -> concat heads -> @ proj_w.T + proj_b

Sharding: data-parallel over batch across 8 cores (4 images/core).

Design (per core):
  - all matmul operands bf16 (PE streams 1 col/cycle regardless of 4B/2B, but
    bf16 halves DMA + SBUF and doubles DVE throughput); PSUM accumulates fp32.
  - sparse attention: the 3x3-window mask on the 24x24 grid makes the score
    matrix banded (halfwidth 25).  With q chunks of 289 and key tiles of 128,
    only 6 of 10 (key-tile x q-chunk) blocks have any unmasked entry:
      ch0 (q 0:289):   kt0, kt1, kt2
      ch1 (q 289:578): kt2, kt3, kt4
    CLS fixups ride along for free:
      * the per-image pad column 577 of x holds a COPY OF TOKEN 0, so the k
        tile col 577 is k(token0): the kt4 stationary is widened to 128 cols
        (65 keys + key0 + zero pad), giving every ch1 query its key-0 column
        (row 65 of the kt4 block), and v row 65 of the last v tile is v0.
      * q0's missing kt3/kt4 contribution lands in ch1's pad column (=q0 copy)
        and is merged into ch0 column 0 with one [65,1] add per head.
      * the merged mask (built host-side per block) kills all double counts.
  - scores run k-major: psum gets mask-free scores, one ACTIVATE exps two
    psum banks into a merged es tile [128, 6*289], one gpsimd multiply applies
    the whole mask.
  - attn@v carries an interleaved ones column so row 64 of each oe psum block
    is the softmax denominator; denominators DMA straight PSUM->SBUF, one
    reciprocal_approx_fast per image computes all 12 heads' inverses, and a
    DRAM round-trip broadcasts them (partition-stride-0 DMA) for the
    normalization multiply.
  - loops are pair-outer / image-inner so qk generation streams 512-wide
    chunks across all 4 images' tokens and the scores->exp->mask->av chain of
    one (img,hi) unit pipelines against its neighbors on deep per-engine
    queues.
"""

import numpy as np
import ml_dtypes

import concourse.bass as bass
import concourse.mybir as mybir
from concourse import bacc
from concourse.bass_utils import run_bass_kernel_spmd
from concourse.tile import TileContext

B, N, C = 32, 577, 768
H, D = 12, 64
NCORES = 8
BPC = B // NCORES            # images per core
NP = N + 1                   # per-image padded token count (col 577 = token 0)
TW = BPC * NP                # 2312 token columns per core
KTW = TW + 64                # kt tile width (zero tail for last image's kt4 aug)
SCALE = float(D) ** -0.5
F32 = mybir.dt.float32
BF16 = mybir.dt.bfloat16
P = 128
QS = 289                     # q chunk size (2 chunks of 289 = 578)
CT = C // P                  # 6 contraction tiles over channels

# v / proj token tiles per image (last includes pad row = token 0)
TT = [(0, 128), (128, 128), (256, 128), (384, 128), (512, 66)]
# score blocks per (pair,img,hi) unit: (k0, q0); stationary is always
# [64, 128] from kt cols k0:k0+128 (kt4 block reads into the next image /
# zero tail, masked off).  es col of block j is j*QS.
BLOCKS = [(0, 0), (128, 0), (256, 0), (256, QS), (384, QS), (512, QS)]
# attn@v chains: (es block idx, v tile idx, K rows)
AV_CH0 = [(0, 0, 128), (1, 1, 128), (2, 2, 128)]
AV_CH1 = [(3, 2, 128), (4, 3, 128), (5, 4, 66)]
GQ = [(0, 512), (512, 512), (1024, 512), (1536, 512), (2048, 264)]  # qk-gen chunks
VCH = [(0, 512), (512, 256)]             # v / proj psum chunks
AF = mybir.ActivationFunctionType
ALU = mybir.AluOpType


def _build_mask_np():
    img = 24
    p = np.arange(img * img)
    pi, pj = p // img, p % img
    ok = (np.abs(pi[:, None] - pi[None, :]) <= 1) & (
        np.abs(pj[:, None] - pj[None, :]) <= 1
    )
    m = np.zeros((N, N), dtype=np.float32)
    m[1:, 1:] = ok
    m[0, :] = True
    m[:, 0] = True
    return m


def _build_merged_mask():
    # maskp[k, q] for the padded/augmented layout described in the docstring
    maskp = np.zeros((640, NP), np.float32)
    maskp[:N, :N] = _build_mask_np()
    maskp[384:N, N] = 1.0    # q0-pad column counts kt3/kt4 keys once (here)
    maskp[N, QS:N] = 1.0     # key0-aug row serves all ch1 queries
    maskp[N, N] = 0.0        # q0 x key0 already counted in (kt0, ch0)
    # hi-major layout: block j of head hi at col (6*hi+j)*QS; the mask is
    # head-independent so the two halves are identical
    mm = np.zeros((P, 12 * QS), np.float32)
    for j, (k0, q0) in enumerate(BLOCKS):
        blk = maskp[k0 : k0 + P, q0 : q0 + QS]
        mm[:, j * QS : (j + 1) * QS] = blk
        mm[:, (6 + j) * QS : (7 + j) * QS] = blk
    return mm


def _bcast_ap(ap1d, parts):
    """1-row AP -> [parts, n] with partition stride 0 (DRAM-source DMA)."""
    return bass.AP(
        tensor=ap1d.tensor, offset=ap1d.offset, ap=[[0, parts]] + list(ap1d.ap)[-1:]
    )


def _build_program(dbg=False):
    nc = bacc.Bacc("TRN2", target_bir_lowering=False, debug=False)
    dbg_t = {}
    if dbg:
        for name, shape, dt in [
            ("dbg_qt", [P, KTW], BF16),
            ("dbg_kt", [P, KTW], BF16),
            ("dbg_es", [P, 12 * QS], BF16),
            ("dbg_den", [2 * H, QS], F32),
            ("dbg_rr", [2 * H, QS], F32),
            ("dbg_oc", [P, NP], BF16),
            ("dbg_oe", [P, 1024], F32),
        ]:
            dbg_t[name] = nc.dram_tensor(name, shape, dt, kind="ExternalOutput").ap()
    xT = nc.dram_tensor("xT", [C, TW], BF16, kind="ExternalInput").ap()
    wqkT = nc.dram_tensor("wqkT", [C, 2 * C], BF16, kind="ExternalInput").ap()
    wvT = nc.dram_tensor("wvT", [C, C], BF16, kind="ExternalInput").ap()
    wpT = nc.dram_tensor("wpT", [C, C], BF16, kind="ExternalInput").ap()
    bqk = nc.dram_tensor("bqk", [2 * C], F32, kind="ExternalInput").ap()
    bv = nc.dram_tensor("bv", [C], F32, kind="ExternalInput").ap()
    bp = nc.dram_tensor("bp", [C], F32, kind="ExternalInput").ap()
    maskm = nc.dram_tensor("maskm", [P, 12 * QS], BF16, kind="ExternalInput").ap()
    y = nc.dram_tensor("y", [BPC * N, C], F32, kind="ExternalOutput").ap()

    with TileContext(nc) as tc:
        with (
            tc.tile_pool(name="singles", bufs=1) as singles,
            tc.tile_pool(name="qkp", bufs=2) as qkp,
            tc.tile_pool(name="esp", bufs=3) as esp,
            tc.tile_pool(name="aux", bufs=3) as aux,
            tc.tile_pool(name="ysp", bufs=2) as ysp,
            tc.tile_pool(name="pall", bufs=2, space="PSUM") as pall,
            tc.tile_pool(name="drp", bufs=2, space="DRAM") as drp,
        ):
            # ---- persistent loads, ordered so compute starts early:
            # img0's x + wv + bv unblock v-gen(0); wqk unblocks qk-gen(0);
            # the rest streams behind.
            def _ct_major(dram, width, dst, cols=None):
                # one DMA: dram [CT*P, width] -> dst [P, CT*width], slice cols
                c0, csz = (0, width) if cols is None else cols
                nc.sync.dma_start(
                    bass.AP(
                        tensor=dst.tensor,
                        offset=dst.offset + c0,
                        ap=[[dst.ap[0][0], P], [width, CT], [1, csz]],
                    ),
                    bass.AP(
                        tensor=dram.tensor,
                        offset=dram.offset + c0,
                        ap=[[width, P], [P * width, CT], [1, csz]],
                    ),
                )

            xs = [
                singles.tile([P, TW], BF16, tag=f"x{ct}", name=f"x{ct}")
                for ct in range(CT)
            ]
            wv_sb = [
                singles.tile([P, C], BF16, tag=f"wv{ct}", name=f"wv{ct}")
                for ct in range(CT)
            ]
            for ct in range(CT):
                nc.sync.dma_start(wv_sb[ct][:], wvT[ct * P : (ct + 1) * P, :])
                nc.scalar.dma_start(
                    xs[ct][:, 0:NP], xT[ct * P : (ct + 1) * P, 0:NP]
                )
            bv_sb = singles.tile([P, C], F32, tag="bv")
            nc.sync.dma_start(bv_sb[:], _bcast_ap(bv, P))
            wqkall = singles.tile([P, CT * 2 * C], BF16, tag="wqkall")
            wqk_sb = [wqkall[:, ct * 2 * C : (ct + 1) * 2 * C] for ct in range(CT)]
            _ct_major(wqkT, 2 * C, wqkall)
            bqk_sb = singles.tile([P, 2 * C // P], F32, tag="bqk")
            nc.sync.dma_start(bqk_sb[:], bqk.rearrange("(o p) -> p o", p=P))
            for ct in range(CT):
                nc.scalar.dma_start(
                    xs[ct][:, NP:TW], xT[ct * P : (ct + 1) * P, NP:TW]
                )
            mask_sb = singles.tile([P, 12 * QS], BF16, tag="mask")
            nc.sync.dma_start(mask_sb[:], maskm)
            wpall = singles.tile([P, CT * C], BF16, tag="wpall")
            wp_sb = [wpall[:, ct * C : (ct + 1) * C] for ct in range(CT)]
            _ct_major(wpT, C, wpall)
            bp_sb = singles.tile([P, C], F32, tag="bp")
            nc.sync.dma_start(bp_sb[:], _bcast_ap(bp, P))

            # per-image state (lives across the whole kernel)
            v_tok = [[None] * len(TT) for _ in range(BPC)]
            oc_sb = [[None] * CT for _ in range(BPC)]
            for b in range(BPC):
                for ct in range(CT):
                    oc_sb[b][ct] = singles.tile(
                        [P, NP], BF16, tag=f"oc{b}_{ct}", name=f"oc{b}_{ct}"
                    )

            def emit_vgen(b):
                for mt, (m0, msz) in enumerate(TT):
                    vt = singles.tile(
                        [P, H, D + 1], BF16, tag=f"vt{b}_{mt}", name=f"vt{b}_{mt}"
                    )
                    v_tok[b][mt] = vt
                    nc.vector.memset(vt[:msz, :, D : D + 1], 1.0)
                    pv = pall.tile([P, C], F32, tag="sc", name="pv")
                    for c0, csz in VCH:
                        for ct in range(CT):
                            nc.tensor.matmul(
                                pv[:msz, c0 : c0 + csz],
                                xs[ct][:, b * NP + m0 : b * NP + m0 + msz],
                                wv_sb[ct][:, c0 : c0 + csz],
                                start=(ct == 0),
                                stop=(ct == CT - 1),
                            )
                    nc.vector.tensor_tensor(
                        vt[:msz, :, 0:D],
                        pv[:msz, :].rearrange("p (h d) -> p h d", d=D),
                        bv_sb[:msz, :].rearrange("p (h d) -> p h d", d=D),
                        ALU.add,
                    )

            def emit_qkgen(hp):
                qt = qkp.tile([P, KTW], BF16, tag="qt")
                kt = qkp.tile([P, KTW], BF16, tag="kt")
                for dst, ft in ((qt, hp), (kt, CT + hp)):
                    for g0, gsz in ((0, 1024), (1024, 1024), (2048, 264)):
                        pg = pall.tile([P, 1024], F32, tag="sc", name="pg")
                        for c0 in range(0, gsz, 512):
                            csz = min(512, gsz - c0)
                            for ct in range(CT):
                                nc.tensor.matmul(
                                    pg[:, c0 : c0 + csz],
                                    wqk_sb[ct][:, ft * P : (ft + 1) * P],
                                    xs[ct][:, g0 + c0 : g0 + c0 + csz],
                                    start=(ct == 0),
                                    stop=(ct == CT - 1),
                                )
                        nc.scalar.activation(
                            dst[:, g0 : g0 + gsz],
                            pg[:, :gsz],
                            AF.Identity,
                            bias=bqk_sb[:, ft : ft + 1],
                        )
                nc.vector.memset(kt[:, TW:KTW], 0.0)
                return qt, kt

            def emit_attn(qt, kt, b, hp):
                # both heads of the pair: score matmuls issue back-to-back
                # with K=64 at PE row groups 0/64 so they stream concurrently
                es = esp.tile([P, 12 * QS], BF16, tag="es")
                for bj, (k0, q0) in enumerate(BLOCKS):
                    sc = pall.tile([P, 1024], F32, tag="sc", name="sc")
                    for hi in range(2):
                        po = 64 * hi
                        nc.tensor.matmul(
                            sc[:, 512 * hi : 512 * hi + QS],
                            kt[po : po + 64, b * NP + k0 : b * NP + k0 + P],
                            qt[po : po + 64, b * NP + q0 : b * NP + q0 + QS],
                            start=True,
                            stop=True,
                        )
                    nc.scalar.activation(
                        es[:].rearrange("p (a c) -> p a c", a=2)[
                            :, :, bj * QS : (bj + 1) * QS
                        ],
                        sc[:].rearrange("p (a c) -> p a c", a=2)[:, :, :QS],
                        AF.Exp,
                    )
                nc.vector.tensor_tensor(es[:], es[:], mask_sb[:], ALU.mult)
                if dbg and hp == 0 and b == 0:
                    nc.sync.dma_start(dbg_t["dbg_es"], es[:])
                den = aux.tile([4, QS], F32, tag="den", bufs=4)
                for hi in range(2):
                    h = 2 * hp + hi
                    oe = pall.tile([P, 1024], F32, tag="oe", name="oe")
                    for chain, col in ((AV_CH0, 0), (AV_CH1, 512)):
                        for i, (bj, vi, krows) in enumerate(chain):
                            nc.tensor.matmul(
                                oe[: D + 1, col : col + QS],
                                v_tok[b][vi][:krows, h, :],
                                es[
                                    :krows,
                                    (6 * hi + bj) * QS : (6 * hi + bj + 1) * QS,
                                ],
                                start=(i == 0),
                                stop=(i == len(chain) - 1),
                            )
                    # denominator row -> SBUF stage -> den_sb rows 2h:2h+2
                    # (q0's kt3/kt4 partial rides in the pad col 577; merge it
                    # into col 0 while the row is still on one partition)
                    dstg = aux.tile([1, NP], F32, tag="dstg")
                    nc.vector.tensor_copy(
                        dstg[:, :].rearrange("p (a c) -> p a c", c=QS),
                        oe[D : D + 1, :].rearrange("p (a c) -> p a c", a=2)[:, :, :QS],
                    )
                    nc.vector.tensor_tensor(
                        dstg[:, 0:1], dstg[:, 0:1], dstg[:, N : N + 1], ALU.add
                    )
                    nc.sync.dma_start(den[2 * hi : 2 * hi + 2, :], dstg[:, :])
                    if hi == 0:
                        t = oc_sb[b][hp][0:D, :]
                    else:
                        t = aux.tile([D, NP], BF16, tag="tmp")
                    nc.vector.tensor_copy(
                        t[:, :].rearrange("p (a c) -> p a c", c=QS),
                        oe[0:D, :].rearrange("p (a c) -> p a c", a=2)[:, :, :QS],
                    )
                    # merge q0's kt3/kt4 contribution (pad col 577) into col 0
                    nc.vector.tensor_tensor(
                        t[:, 0:1], t[:, 0:1], t[:, N : N + 1], ALU.add
                    )
                    if hi == 1:
                        nc.sync.dma_start(oc_sb[b][hp][D:P, :], t[:, :])
                return den

            def emit_norm_pair(b, hp, den):
                # normalize this pair's oc as soon as its denominators land;
                # for pairs 0..4 the whole chain hides behind later pairs
                rr = aux.tile([4, QS], F32, tag="rr")
                if dbg and b == 0 and hp == 0:
                    nc.sync.dma_start(dbg_t["dbg_den"][0:4, :], den[:])
                nc.vector.reciprocal_approx_fast(rr[:], den[:])
                rrb = aux.tile([4, QS], BF16, tag="rrb")
                nc.vector.tensor_copy(rrb[:], rr[:])
                rrd = drp.tile([4, QS], BF16, tag="rrd")
                nc.sync.dma_start(rrd[:], rrb[:])
                rb = aux.tile([P, NP], BF16, tag="rb")
                nc.sync.dma_start(
                    rb[:, :],
                    bass.AP(
                        tensor=rrd.tensor,
                        offset=rrd.offset,
                        ap=[[NP, 2], [0, 64], [1, NP]],
                    ),
                )
                nc.gpsimd.tensor_tensor(
                    oc_sb[b][hp][:], oc_sb[b][hp][:], rb[:], ALU.mult
                )

            def emit_proj(b):
                for m0, msz in TT:
                    pp = pall.tile([P, C], F32, tag="sc", name="pp")
                    for c0, csz in VCH:
                        for ct in range(CT):
                            nc.tensor.matmul(
                                pp[:msz, c0 : c0 + csz],
                                oc_sb[b][ct][:, m0 : m0 + msz],
                                wp_sb[ct][:, c0 : c0 + csz],
                                start=(ct == 0),
                                stop=(ct == CT - 1),
                            )
                    ysb = ysp.tile([P, C], F32, tag="ysb", name="ysb")
                    nc.vector.tensor_tensor(
                        ysb[:msz, :], pp[:msz, :], bp_sb[:msz, :], ALU.add
                    )
                    mo = min(msz, N - m0)
                    nc.sync.dma_start(y[b * N + m0 : b * N + m0 + mo, :], ysb[:mo, :])

            # ---- schedule ----
            emit_vgen(0)
            qt, kt = emit_qkgen(0)
            if dbg:
                nc.sync.dma_start(dbg_t["dbg_qt"], qt[:])
                nc.sync.dma_start(dbg_t["dbg_kt"], kt[:])
            pending = None
            for hp in range(H // 2):
                for b in range(BPC):
                    den = emit_attn(qt, kt, b, hp)
                    if pending is not None:
                        emit_norm_pair(*pending)
                    pending = (b, hp, den)
                    if hp == 0 and b + 1 < BPC:
                        emit_vgen(b + 1)
                    if hp == H // 2 - 1 and b >= 1:
                        emit_proj(b - 1)
                if hp + 1 < H // 2:
                    qt, kt = emit_qkgen(hp + 1)
            emit_norm_pair(*pending)
            emit_proj(BPC - 1)
            if dbg:
                nc.sync.dma_start(dbg_t["dbg_oc"], oc_sb[0][0][:])

    nc.finalize()
    return nc


_CACHE = {}


def _make_in_maps(x, qkv_w, qkv_b, proj_w, proj_b):
    bf = ml_dtypes.bfloat16
    x = np.asarray(x, np.float32)
    qkv_w = np.asarray(qkv_w, np.float32)
    qkv_b = np.asarray(qkv_b, np.float32)
    proj_w = np.asarray(proj_w, np.float32)
    proj_b = np.asarray(proj_b, np.float32)

    wqk_h = qkv_w[: 2 * C].copy()
    wqk_h[:C] *= SCALE
    wqkT = np.ascontiguousarray(wqk_h.T.astype(bf))
    wvT = np.ascontiguousarray(qkv_w[2 * C :].T.astype(bf))
    wpT = np.ascontiguousarray(proj_w.T.astype(bf))
    bqk_h = qkv_b[: 2 * C].copy()
    bqk_h[:C] *= SCALE
    bv_h = np.ascontiguousarray(qkv_b[2 * C :])
    maskm = np.ascontiguousarray(_build_merged_mask().astype(bf))

    in_maps = []
    for c in range(NCORES):
        xp_c = np.zeros((BPC, NP, C), np.float32)
        xp_c[:, :N, :] = x[c * BPC : (c + 1) * BPC]
        xp_c[:, N, :] = x[c * BPC : (c + 1) * BPC, 0, :]
        xT_c = np.ascontiguousarray(xp_c.reshape(TW, C).T.astype(bf))
        in_maps.append(
            {
                "xT": xT_c,
                "wqkT": wqkT,
                "wvT": wvT,
                "wpT": wpT,
                "bqk": bqk_h,
                "bv": bv_h,
                "bp": proj_b,
                "maskm": maskm,
            }
        )
    return in_maps


def kernel(x, qkv_w, qkv_b, proj_w, proj_b):
    if "nc" not in _CACHE:
        _CACHE["nc"] = _build_program()
    nc = _CACHE["nc"]

    in_maps = _make_in_maps(x, qkv_w, qkv_b, proj_w, proj_b)
    res = run_bass_kernel_spmd(nc, in_maps, list(range(NCORES)))
    out = np.concatenate(
        [res.results[c]["y"].reshape(BPC, N, C) for c in range(NCORES)], axis=0
    )
    return out.astype(np.float32)


# revision 27
# speedup vs baseline: 1.0639x; 1.0639x over previous
"""Trainium2 Bass kernel for CustomAttention (ViT-style windowed attention).

Math (per batch element):
  qkv = x @ qkv_w.T + qkv_b            -> q, k, v  [H=12 heads, D=64]
  s   = (q * D^-0.5) @ k.T             masked by a fixed 24x24-grid window
  attn = softmax(s)                    (CLS row/col always attended)
  out  = attn @ v                      -> concat heads -> @ proj_w.T + proj_b

Sharding: data-parallel over batch across 8 cores (4 images/core).

Design (per core):
  - all matmul operands bf16 (PE streams 1 col/cycle regardless of 4B/2B, but
    bf16 halves DMA + SBUF and doubles DVE throughput); PSUM accumulates fp32.
  - sparse attention: the 3x3-window mask on the 24x24 grid makes the score
    matrix banded (halfwidth 25).  With q chunks of 289 and key tiles of 128,
    only 6 of 10 (key-tile x q-chunk) blocks have any unmasked entry:
      ch0 (q 0:289):   kt0, kt1, kt2
      ch1 (q 289:578): kt2, kt3, kt4
    CLS fixups ride along for free:
      * the per-image pad column 577 of x holds a COPY OF TOKEN 0, so the k
        tile col 577 is k(token0): the kt4 stationary is widened to 128 cols
        (65 keys + key0 + zero pad), giving every ch1 query its key-0 column
        (row 65 of the kt4 block), and v row 65 of the last v tile is v0.
      * q0's missing kt3/kt4 contribution lands in ch1's pad column (=q0 copy)
        and is merged into ch0 column 0 with one [65,1] add per head.
      * the merged mask (built host-side per block) kills all double counts.
  - scores run k-major: psum gets mask-free scores, one ACTIVATE exps two
    psum banks into a merged es tile [128, 6*289], one gpsimd multiply applies
    the whole mask.
  - attn@v carries an interleaved ones column so row 64 of each oe psum block
    is the softmax denominator; denominators DMA straight PSUM->SBUF, one
    reciprocal_approx_fast per image computes all 12 heads' inverses, and a
    DRAM round-trip broadcasts them (partition-stride-0 DMA) for the
    normalization multiply.
  - loops are pair-outer / image-inner so qk generation streams 512-wide
    chunks across all 4 images' tokens and the scores->exp->mask->av chain of
    one (img,hi) unit pipelines against its neighbors on deep per-engine
    queues.
"""

import numpy as np
import ml_dtypes

import concourse.bass as bass
import concourse.mybir as mybir
from concourse import bacc
from concourse.bass_utils import run_bass_kernel_spmd
from concourse.tile import TileContext

B, N, C = 32, 577, 768
H, D = 12, 64
NCORES = 8
BPC = B // NCORES            # images per core
NP = N + 1                   # per-image padded token count (col 577 = token 0)
TW = BPC * NP                # 2312 token columns per core
KTW = TW + 64                # kt tile width (zero tail for last image's kt4 aug)
SCALE = float(D) ** -0.5
F32 = mybir.dt.float32
BF16 = mybir.dt.bfloat16
P = 128
QS = 289                     # q chunk size (2 chunks of 289 = 578)
CT = C // P                  # 6 contraction tiles over channels

# v / proj token tiles per image (last includes pad row = token 0)
TT = [(0, 128), (128, 128), (256, 128), (384, 128), (512, 66)]
# score blocks per (pair,img,hi) unit: (k0, q0); stationary is always
# [64, 128] from kt cols k0:k0+128 (kt4 block reads into the next image /
# zero tail, masked off).  es col of block j is j*QS.
BLOCKS = [(0, 0), (128, 0), (256, 0), (256, QS), (384, QS), (512, QS)]
# attn@v chains: (es block idx, v tile idx, K rows)
AV_CH0 = [(0, 0, 128), (1, 1, 128), (2, 2, 128)]
AV_CH1 = [(3, 2, 128), (4, 3, 128), (5, 4, 66)]
GQ = [(0, 512), (512, 512), (1024, 512), (1536, 512), (2048, 264)]  # qk-gen chunks
VCH = [(0, 512), (512, 256)]             # v / proj psum chunks
AF = mybir.ActivationFunctionType
ALU = mybir.AluOpType


def _build_mask_np():
    img = 24
    p = np.arange(img * img)
    pi, pj = p // img, p % img
    ok = (np.abs(pi[:, None] - pi[None, :]) <= 1) & (
        np.abs(pj[:, None] - pj[None, :]) <= 1
    )
    m = np.zeros((N, N), dtype=np.float32)
    m[1:, 1:] = ok
    m[0, :] = True
    m[:, 0] = True
    return m


def _build_merged_mask():
    # maskp[k, q] for the padded/augmented layout described in the docstring
    maskp = np.zeros((640, NP), np.float32)
    maskp[:N, :N] = _build_mask_np()
    maskp[384:N, N] = 1.0    # q0-pad column counts kt3/kt4 keys once (here)
    maskp[N, QS:N] = 1.0     # key0-aug row serves all ch1 queries
    maskp[N, N] = 0.0        # q0 x key0 already counted in (kt0, ch0)
    # hi-major layout: block j of head hi at col (6*hi+j)*QS; the mask is
    # head-independent so the two halves are identical
    mm = np.zeros((P, 12 * QS), np.float32)
    for j, (k0, q0) in enumerate(BLOCKS):
        blk = maskp[k0 : k0 + P, q0 : q0 + QS]
        mm[:, j * QS : (j + 1) * QS] = blk
        mm[:, (6 + j) * QS : (7 + j) * QS] = blk
    return mm


def _bcast_ap(ap1d, parts):
    """1-row AP -> [parts, n] with partition stride 0 (DRAM-source DMA)."""
    return bass.AP(
        tensor=ap1d.tensor, offset=ap1d.offset, ap=[[0, parts]] + list(ap1d.ap)[-1:]
    )


def _build_program(dbg=False):
    nc = bacc.Bacc("TRN2", target_bir_lowering=False, debug=False)
    dbg_t = {}
    if dbg:
        for name, shape, dt in [
            ("dbg_qt", [P, KTW], BF16),
            ("dbg_kt", [P, KTW], BF16),
            ("dbg_es", [P, 12 * QS], BF16),
            ("dbg_den", [2 * H, QS], F32),
            ("dbg_rr", [2 * H, QS], F32),
            ("dbg_oc", [P, NP], BF16),
            ("dbg_oe", [P, 1024], F32),
        ]:
            dbg_t[name] = nc.dram_tensor(name, shape, dt, kind="ExternalOutput").ap()
    xT = nc.dram_tensor("xT", [C, TW], BF16, kind="ExternalInput").ap()
    wqkT = nc.dram_tensor("wqkT", [C, 2 * C], BF16, kind="ExternalInput").ap()
    wvT = nc.dram_tensor("wvT", [C, C], BF16, kind="ExternalInput").ap()
    wpT = nc.dram_tensor("wpT", [C, C], BF16, kind="ExternalInput").ap()
    bqk = nc.dram_tensor("bqk", [2 * C], F32, kind="ExternalInput").ap()
    bv = nc.dram_tensor("bv", [C], F32, kind="ExternalInput").ap()
    bp = nc.dram_tensor("bp", [C], F32, kind="ExternalInput").ap()
    maskm = nc.dram_tensor("maskm", [P, 12 * QS], BF16, kind="ExternalInput").ap()
    y = nc.dram_tensor("y", [BPC * N, C], F32, kind="ExternalOutput").ap()

    with TileContext(nc) as tc:
        with (
            tc.tile_pool(name="singles", bufs=1) as singles,
            tc.tile_pool(name="qkp", bufs=2) as qkp,
            tc.tile_pool(name="esp", bufs=3) as esp,
            tc.tile_pool(name="aux", bufs=3) as aux,
            tc.tile_pool(name="ysp", bufs=2) as ysp,
            tc.tile_pool(name="pall", bufs=2, space="PSUM") as pall,
            tc.tile_pool(name="drp", bufs=2, space="DRAM") as drp,
        ):
            # ---- persistent loads, ordered so compute starts early:
            # img0's x + wv + bv unblock v-gen(0); wqk unblocks qk-gen(0);
            # the rest streams behind.
            def _ct_major(dram, width, dst, cols=None):
                # one DMA: dram [CT*P, width] -> dst [P, CT*width], slice cols
                c0, csz = (0, width) if cols is None else cols
                nc.sync.dma_start(
                    bass.AP(
                        tensor=dst.tensor,
                        offset=dst.offset + c0,
                        ap=[[dst.ap[0][0], P], [width, CT], [1, csz]],
                    ),
                    bass.AP(
                        tensor=dram.tensor,
                        offset=dram.offset + c0,
                        ap=[[width, P], [P * width, CT], [1, csz]],
                    ),
                )

            xs = [
                singles.tile([P, TW], BF16, tag=f"x{ct}", name=f"x{ct}")
                for ct in range(CT)
            ]
            wv_sb = [
                singles.tile([P, C], BF16, tag=f"wv{ct}", name=f"wv{ct}")
                for ct in range(CT)
            ]
            engs = [nc.sync, nc.scalar, nc.gpsimd]
            for ct in range(CT):
                engs[ct % 3].dma_start(wv_sb[ct][:], wvT[ct * P : (ct + 1) * P, :])
                engs[(ct + 1) % 3].dma_start(
                    xs[ct][:, 0:NP], xT[ct * P : (ct + 1) * P, 0:NP]
                )
            bv_sb = singles.tile([P, C], F32, tag="bv")
            nc.sync.dma_start(bv_sb[:], _bcast_ap(bv, P))
            wqkall = singles.tile([P, CT * 2 * C], BF16, tag="wqkall")
            wqk_sb = [wqkall[:, ct * 2 * C : (ct + 1) * 2 * C] for ct in range(CT)]
            _ct_major(wqkT, 2 * C, wqkall)
            bqk_sb = singles.tile([P, 2 * C // P], F32, tag="bqk")
            nc.sync.dma_start(bqk_sb[:], bqk.rearrange("(o p) -> p o", p=P))
            for ct in range(CT):
                nc.scalar.dma_start(
                    xs[ct][:, NP:TW], xT[ct * P : (ct + 1) * P, NP:TW]
                )
            mask_sb = singles.tile([P, 12 * QS], BF16, tag="mask")
            nc.sync.dma_start(mask_sb[:], maskm)
            wpall = singles.tile([P, CT * C], BF16, tag="wpall")
            wp_sb = [wpall[:, ct * C : (ct + 1) * C] for ct in range(CT)]
            _ct_major(wpT, C, wpall)
            bp_sb = singles.tile([P, C], F32, tag="bp")
            nc.sync.dma_start(bp_sb[:], _bcast_ap(bp, P))

            # per-image state (lives across the whole kernel)
            v_tok = [[None] * len(TT) for _ in range(BPC)]
            oc_sb = [[None] * CT for _ in range(BPC)]
            for b in range(BPC):
                for ct in range(CT):
                    oc_sb[b][ct] = singles.tile(
                        [P, NP], BF16, tag=f"oc{b}_{ct}", name=f"oc{b}_{ct}"
                    )

            def emit_vgen(b):
                for mt, (m0, msz) in enumerate(TT):
                    vt = singles.tile(
                        [P, H, D + 1], BF16, tag=f"vt{b}_{mt}", name=f"vt{b}_{mt}"
                    )
                    v_tok[b][mt] = vt
                    nc.vector.memset(vt[:msz, :, D : D + 1], 1.0)
                    pv = pall.tile([P, C], F32, tag="sc", name="pv")
                    for c0, csz in VCH:
                        for ct in range(CT):
                            nc.tensor.matmul(
                                pv[:msz, c0 : c0 + csz],
                                xs[ct][:, b * NP + m0 : b * NP + m0 + msz],
                                wv_sb[ct][:, c0 : c0 + csz],
                                start=(ct == 0),
                                stop=(ct == CT - 1),
                            )
                    nc.vector.tensor_tensor(
                        vt[:msz, :, 0:D],
                        pv[:msz, :].rearrange("p (h d) -> p h d", d=D),
                        bv_sb[:msz, :].rearrange("p (h d) -> p h d", d=D),
                        ALU.add,
                    )

            def emit_qkgen(hp):
                qt = qkp.tile([P, KTW], BF16, tag="qt")
                kt = qkp.tile([P, KTW], BF16, tag="kt")
                for dst, ft in ((qt, hp), (kt, CT + hp)):
                    for g0, gsz in ((0, 1024), (1024, 1024), (2048, 264)):
                        pg = pall.tile([P, 1024], F32, tag="sc", name="pg")
                        for c0 in range(0, gsz, 512):
                            csz = min(512, gsz - c0)
                            for ct in range(CT):
                                nc.tensor.matmul(
                                    pg[:, c0 : c0 + csz],
                                    wqk_sb[ct][:, ft * P : (ft + 1) * P],
                                    xs[ct][:, g0 + c0 : g0 + c0 + csz],
                                    start=(ct == 0),
                                    stop=(ct == CT - 1),
                                )
                        nc.scalar.activation(
                            dst[:, g0 : g0 + gsz],
                            pg[:, :gsz],
                            AF.Identity,
                            bias=bqk_sb[:, ft : ft + 1],
                        )
                nc.vector.memset(kt[:, TW:KTW], 0.0)
                return qt, kt

            def emit_attn(qt, kt, b, hp):
                # both heads of the pair: score matmuls issue back-to-back
                # with K=64 at PE row groups 0/64 so they stream concurrently
                es = esp.tile([P, 12 * QS], BF16, tag="es")
                for bj, (k0, q0) in enumerate(BLOCKS):
                    sc = pall.tile([P, 1024], F32, tag="sc", name="sc")
                    for hi in range(2):
                        po = 64 * hi
                        nc.tensor.matmul(
                            sc[:, 512 * hi : 512 * hi + QS],
                            kt[po : po + 64, b * NP + k0 : b * NP + k0 + P],
                            qt[po : po + 64, b * NP + q0 : b * NP + q0 + QS],
                            start=True,
                            stop=True,
                        )
                    nc.scalar.activation(
                        es[:].rearrange("p (a c) -> p a c", a=2)[
                            :, :, bj * QS : (bj + 1) * QS
                        ],
                        sc[:].rearrange("p (a c) -> p a c", a=2)[:, :, :QS],
                        AF.Exp,
                    )
                nc.vector.tensor_tensor(es[:], es[:], mask_sb[:], ALU.mult)
                if dbg and hp == 0 and b == 0:
                    nc.sync.dma_start(dbg_t["dbg_es"], es[:])
                den = aux.tile([4, QS], F32, tag="den", bufs=4)
                for hi in range(2):
                    h = 2 * hp + hi
                    oe = pall.tile([P, 1024], F32, tag="oe", name="oe")
                    for chain, col in ((AV_CH0, 0), (AV_CH1, 512)):
                        for i, (bj, vi, krows) in enumerate(chain):
                            nc.tensor.matmul(
                                oe[: D + 1, col : col + QS],
                                v_tok[b][vi][:krows, h, :],
                                es[
                                    :krows,
                                    (6 * hi + bj) * QS : (6 * hi + bj + 1) * QS,
                                ],
                                start=(i == 0),
                                stop=(i == len(chain) - 1),
                            )
                    # denominator row -> SBUF stage -> den_sb rows 2h:2h+2
                    # (q0's kt3/kt4 partial rides in the pad col 577; merge it
                    # into col 0 while the row is still on one partition)
                    dstg = aux.tile([1, NP], F32, tag="dstg")
                    nc.vector.tensor_copy(
                        dstg[:, :].rearrange("p (a c) -> p a c", c=QS),
                        oe[D : D + 1, :].rearrange("p (a c) -> p a c", a=2)[:, :, :QS],
                    )
                    nc.vector.tensor_tensor(
                        dstg[:, 0:1], dstg[:, 0:1], dstg[:, N : N + 1], ALU.add
                    )
                    nc.sync.dma_start(den[2 * hi : 2 * hi + 2, :], dstg[:, :])
                    if hi == 0:
                        t = oc_sb[b][hp][0:D, :]
                    else:
                        t = aux.tile([D, NP], BF16, tag="tmp")
                    nc.vector.tensor_copy(
                        t[:, :].rearrange("p (a c) -> p a c", c=QS),
                        oe[0:D, :].rearrange("p (a c) -> p a c", a=2)[:, :, :QS],
                    )
                    # merge q0's kt3/kt4 contribution (pad col 577) into col 0
                    nc.vector.tensor_tensor(
                        t[:, 0:1], t[:, 0:1], t[:, N : N + 1], ALU.add
                    )
                    if hi == 1:
                        nc.sync.dma_start(oc_sb[b][hp][D:P, :], t[:, :])
                return den

            def emit_norm_pair(b, hp, den):
                # normalize this pair's oc as soon as its denominators land;
                # for pairs 0..4 the whole chain hides behind later pairs
                rr = aux.tile([4, QS], F32, tag="rr")
                if dbg and b == 0 and hp == 0:
                    nc.sync.dma_start(dbg_t["dbg_den"][0:4, :], den[:])
                nc.vector.reciprocal_approx_fast(rr[:], den[:])
                rrb = aux.tile([4, QS], BF16, tag="rrb")
                nc.vector.tensor_copy(rrb[:], rr[:])
                rrd = drp.tile([4, QS], BF16, tag="rrd")
                nc.sync.dma_start(rrd[:], rrb[:])
                rb = aux.tile([P, NP], BF16, tag="rb")
                nc.sync.dma_start(
                    rb[:, :],
                    bass.AP(
                        tensor=rrd.tensor,
                        offset=rrd.offset,
                        ap=[[NP, 2], [0, 64], [1, NP]],
                    ),
                )
                nc.gpsimd.tensor_tensor(
                    oc_sb[b][hp][:], oc_sb[b][hp][:], rb[:], ALU.mult
                )

            def emit_proj(b):
                for m0, msz in TT:
                    pp = pall.tile([P, C], F32, tag="sc", name="pp")
                    for c0, csz in VCH:
                        for ct in range(CT):
                            nc.tensor.matmul(
                                pp[:msz, c0 : c0 + csz],
                                oc_sb[b][ct][:, m0 : m0 + msz],
                                wp_sb[ct][:, c0 : c0 + csz],
                                start=(ct == 0),
                                stop=(ct == CT - 1),
                            )
                    ysb = ysp.tile([P, C], F32, tag="ysb", name="ysb")
                    nc.vector.tensor_tensor(
                        ysb[:msz, :], pp[:msz, :], bp_sb[:msz, :], ALU.add
                    )
                    mo = min(msz, N - m0)
                    nc.sync.dma_start(y[b * N + m0 : b * N + m0 + mo, :], ysb[:mo, :])

            # ---- schedule ----
            emit_vgen(0)
            qt, kt = emit_qkgen(0)
            if dbg:
                nc.sync.dma_start(dbg_t["dbg_qt"], qt[:])
                nc.sync.dma_start(dbg_t["dbg_kt"], kt[:])
            pending = None
            for hp in range(H // 2):
                for b in range(BPC):
                    den = emit_attn(qt, kt, b, hp)
                    if pending is not None:
                        emit_norm_pair(*pending)
                    pending = (b, hp, den)
                    if hp == 0 and b + 1 < BPC:
                        emit_vgen(b + 1)
                    if hp == H // 2 - 1 and b >= 2:
                        emit_proj(b - 2)
                if hp + 1 < H // 2:
                    qt, kt = emit_qkgen(hp + 1)
            emit_norm_pair(*pending)
            emit_proj(BPC - 2)
            emit_proj(BPC - 1)
            if dbg:
                nc.sync.dma_start(dbg_t["dbg_oc"], oc_sb[0][0][:])

    nc.finalize()
    return nc


_CACHE = {}


def _make_in_maps(x, qkv_w, qkv_b, proj_w, proj_b):
    bf = ml_dtypes.bfloat16
    x = np.asarray(x, np.float32)
    qkv_w = np.asarray(qkv_w, np.float32)
    qkv_b = np.asarray(qkv_b, np.float32)
    proj_w = np.asarray(proj_w, np.float32)
    proj_b = np.asarray(proj_b, np.float32)

    wqk_h = qkv_w[: 2 * C].copy()
    wqk_h[:C] *= SCALE
    wqkT = np.ascontiguousarray(wqk_h.T.astype(bf))
    wvT = np.ascontiguousarray(qkv_w[2 * C :].T.astype(bf))
    wpT = np.ascontiguousarray(proj_w.T.astype(bf))
    bqk_h = qkv_b[: 2 * C].copy()
    bqk_h[:C] *= SCALE
    bv_h = np.ascontiguousarray(qkv_b[2 * C :])
    maskm = np.ascontiguousarray(_build_merged_mask().astype(bf))

    in_maps = []
    for c in range(NCORES):
        xp_c = np.zeros((BPC, NP, C), np.float32)
        xp_c[:, :N, :] = x[c * BPC : (c + 1) * BPC]
        xp_c[:, N, :] = x[c * BPC : (c + 1) * BPC, 0, :]
        xT_c = np.ascontiguousarray(xp_c.reshape(TW, C).T.astype(bf))
        in_maps.append(
            {
                "xT": xT_c,
                "wqkT": wqkT,
                "wvT": wvT,
                "wpT": wpT,
                "bqk": bqk_h,
                "bv": bv_h,
                "bp": proj_b,
                "maskm": maskm,
            }
        )
    return in_maps


def kernel(x, qkv_w, qkv_b, proj_w, proj_b):
    if "nc" not in _CACHE:
        _CACHE["nc"] = _build_program()
    nc = _CACHE["nc"]

    in_maps = _make_in_maps(x, qkv_w, qkv_b, proj_w, proj_b)
    res = run_bass_kernel_spmd(nc, in_maps, list(range(NCORES)))
    out = np.concatenate(
        [res.results[c]["y"].reshape(BPC, N, C) for c in range(NCORES)], axis=0
    )
    return out.astype(np.float32)
